# revision 36
# baseline (speedup 1.0000x reference)
"""Sliding-window GQA attention (B=2,T=2048,D=2048,N=8,K=4,H=256,W=1024) on 8 trn2 cores.

Sharding: batch over 2 (fsdp) x heads over 4 (tp). Core (b, tp) computes 2 q heads /
1 kv head for batch b; partial [T, D] outputs are summed over tp on the host.

All matmuls are bf16 x bf16 accumulating in f32 PSUM; bf16 ap-128 stays full-rate,
which lets phase B skip the guaranteed-masked halves of the two window-edge key
blocks. Inputs ship bf16, host-packed so every DMA moves contiguous >=1KB rows:
x halves on the SP ring (first tiles split for cold-queue parallelism), weights/
tables on the ACT ring in first-use order (k-parts and v-weight halves early).

PE-work minimization (vs the 277us revision):
  - softmax denominators: DVE pairwise-add tree over the masked exp tiles
    (all-bf16 SBUF tensor_tensor = fast DVE mode) + ONE ones-matmul per pair
    (512 rows) instead of per-key-block ones-matmuls (4608 rows/pair).
  - RMS sum-of-squares: DVE pre-adds the squared h-halves, single ones-matmul.
  - 1/den via nc.vector.reciprocal_approx_fast, so phase B's ACT runs Exp only:
    the Exp<->Abs_reciprocal_sqrt activation-table reloads (1.28us, 2/pair) are
    gone; both tables are pre-loaded via dummy ops off the critical path.
PE-idle minimization:
  - q/k PSUM banks are staged to SBUF bf16 on ACT right after the squares (rms
    (1+scale) folded into the copy); the RoPE rotation is pure bf16
    tensor_tensor on DVE reading staged copies, so projections never wait on
    the rotation chains.  The Pool engine is left IDLE on purpose: DVE and
    GpSimd share SBUF ports, and any Pool activity doubles DVE op latency.
  - one unified PSUM pool: phase B's tiles reuse phase A's bank tags, so B's
    first matmuls wait on one early-freed bank, not a pool-release barrier.
  - last quarter: q-head pss reductions ride the freed pk banks and the Exp
    table swap happens while v1 still streams, so B starts with the seam dry.
  - phase B runs software-pipelined: logits(p) | tail(p-1) | den-tree(p) |
    o-proj(p-2); o-proj casts on ACT; out-DMAs alternate SP/ACT rings, final
    block split per half so the kernel tail is one small transfer.
"""
import os

import numpy as np
import ml_dtypes

import concourse.bacc as bacc
import concourse.mybir as mybir
from concourse.tile import TileContext
from concourse.bass_utils import run_bass_kernel_spmd

try:  # pragma: no cover - profiling hook is optional
    from antenv.axon_hooks import get_axon_ntff_profile_hook  # noqa: F401
except ImportError:
    os.environ.setdefault("BASS_NEVER_TRACE", "1")


F32 = mybir.dt.float32
BF16 = mybir.dt.bfloat16
AF = mybir.ActivationFunctionType
OP = mybir.AluOpType

B, T, D = 2, 2048, 2048
N, KV, H = 8, 4, 256
WINDOW = 1024
BASE_FREQ = 10000.0
EPS = 1e-6
NB = T // 128          # 16 token blocks
NQ = 4                 # t quarters for projections (512 each)
NPAIR = 8              # query-block pairs (256 tokens each)
NPB = np.dtype(ml_dtypes.bfloat16)


def _mask_idx(i, j):
    if j == i + 1:
        return 3
    if j == i:
        return 2
    if j == i - 7:
        return 1
    if j == i - 8:
        return 0
    return None


def _jlist(i):
    return list(range(max(0, i - 8), i + 2))


def _span(i, j):
    # query half actually alive for this key block (128-granular)
    if j == i + 1:
        return 128, 256
    if j == i - 8:
        return 0, 128
    return 0, 256


def _slots(pi):
    # E-tile slot per key block: full-span blocks pack into slots 0..nf-1
    # (so the DVE add-tree reduces them with strided ops), lo-edge -> 8,
    # hi-edge -> 9.
    i = 2 * pi
    js = _jlist(i)
    slot, nf = {}, 0
    for j in js:
        lo, hi = _span(i, j)
        if (lo, hi) == (0, 256):
            slot[j] = nf
            nf += 1
        elif (lo, hi) == (0, 128):
            slot[j] = 8
        else:
            slot[j] = 9
    return js, slot, nf


def _build():
    nc = bacc.Bacc(None)

    xh = nc.dram_tensor("xh", [2, 16, 128, 1024], BF16, kind="ExternalInput")
    # fused k/q weights, kind-major: [c, 128, 3, 2, 256]; kind 0=kw, 1/2=qw
    wA = nc.dram_tensor("wA", [8, 128, 3, 2, 256], BF16, kind="ExternalInput")
    vw = nc.dram_tensor("vw", [128, 16, 256], BF16, kind="ExternalInput")
    ow = nc.dram_tensor("ow", [2, 2, 128, D], BF16, kind="ExternalInput")  # [nl,hh,128,D]
    cst = nc.dram_tensor("cst", [4, 128, 1024], BF16, kind="ExternalInput")
    masks = nc.dram_tensor("masks", [128, 4, 256], BF16, kind="ExternalInput")
    scs = nc.dram_tensor("scs", [128, 2, 2], F32, kind="ExternalInput")  # (1+scale)[q/k][hh]
    out = nc.dram_tensor("out", [16, 128, D], BF16, kind="ExternalOutput")

    with TileContext(nc) as tc:
        # one PSUM pool for the whole kernel: phase B reuses phase A's banks
        # by tag, so B's first matmuls wait only on the bank they reuse
        # (freed early) instead of a pool-release barrier over every psA user
        with tc.tile_pool(name="pers", bufs=1) as pers, \
             tc.tile_pool(name="ps", bufs=1, space="PSUM") as psum:
            kT_sb = pers.tile([128, 2, T], BF16)
            v_sb = pers.tile([128, NB, H], BF16)
            qT_sb = pers.tile([128, 2, 2, T], BF16)
            scs_sb = pers.tile([128, 2, 2], F32)
            ones32 = pers.tile([128, 128], F32)
            ones_b = pers.tile([128, 128], BF16)
            bias_q = pers.tile([128, 1], F32)
            bias_k = pers.tile([128, 1], F32)
            dum = pers.tile([128, 1], F32)
            masks_sb = pers.tile([128, 4, 256], BF16)
            ow_sb = pers.tile([128, 2, 2, D], BF16)

            nc.vector.memset(ones32, 1.0)
            nc.vector.tensor_copy(ones_b, ones32)
            nc.vector.memset(bias_q, float(H * EPS))
            nc.vector.memset(bias_k, EPS)

            # ---------------- Phase A: all projections + rms + rope ----------------
            with tc.tile_pool(name="wts", bufs=1) as wts, \
                 tc.tile_pool(name="xs", bufs=24) as xs, \
                 tc.tile_pool(name="ropep", bufs=1) as ropep:

                # ACT-ring DMAs in first-use order (x rides the SP ring).
                # k-parts of the fused weights ship first so k_mm(0) starts asap.
                wA_t = {}
                vw_sb = wts.tile([128, 16, 256], BF16)
                for c in range(8):
                    wA_t[c] = wts.tile([128, 3, 2, 256], BF16, tag=f"wA{c}", name=f"wA{c}")
                    if c == 0:
                        # finest first: cold DMA queues run at per-queue
                        # bandwidth, so the first matmul's weights ship alone
                        nc.scalar.dma_start(out=wA_t[c][:, 0, 0], in_=wA[c, :, 0, 0])
                        nc.scalar.dma_start(out=wA_t[c][:, 0, 1], in_=wA[c, :, 0, 1])
                        nc.scalar.dma_start(out=wA_t[c][:, 1], in_=wA[c, :, 1])
                        nc.scalar.dma_start(out=wA_t[c][:, 2], in_=wA[c, :, 2])
                    elif c == 1:
                        for kd in range(3):
                            nc.scalar.dma_start(out=wA_t[c][:, kd], in_=wA[c, :, kd])
                    elif c < 6:
                        nc.scalar.dma_start(out=wA_t[c][:, 0], in_=wA[c, :, 0])
                        nc.scalar.dma_start(out=wA_t[c][:, 1:3], in_=wA[c, :, 1:3])
                    else:
                        nc.scalar.dma_start(out=wA_t[c], in_=wA[c])
                    if c in (3, 5):
                        # v weights ride between the k/q tiles in two halves so
                        # they land well before quarter 0's v matmuls (a single
                        # 1MB transfer sits on one ~46GB/s queue for >20us)
                        h = 8 * (c == 5)
                        nc.scalar.dma_start(out=vw_sb[:, h:h + 8],
                                            in_=vw[:, h:h + 8])
                cst_sb = {}
                for qt in range(NQ):
                    cst_sb[qt] = ropep.tile([128, 1024], BF16, tag=f"cst{qt}", name=f"cst{qt}")
                    nc.scalar.dma_start(out=cst_sb[qt], in_=cst[qt, :, :])
                    if qt == 0:
                        nc.scalar.dma_start(out=scs_sb, in_=scs[:, :, :])
                        # pre-load the abs_rsqrt table once descriptors are out
                        nc.scalar.activation(dum, ones32[:, 0:1],
                                             AF.Abs_reciprocal_sqrt,
                                             scale=1.0, bias=bias_k)

                # x halves on the SP ring; the first four tiles ship as
                # 128KB halves so the cold (per-queue-bandwidth-limited) DMA
                # engines overlap and quarter 0 is never starved.
                xh_t = {}
                for half in range(2):
                    for d in range(16):
                        xt = xs.tile([128, 1024], BF16, tag="xt")
                        if half == 0 and d == 0:
                            nc.sync.dma_start(out=xt[:, 0:256], in_=xh[half, d, :, 0:256])
                            nc.sync.dma_start(out=xt[:, 256:512], in_=xh[half, d, :, 256:512])
                            nc.sync.dma_start(out=xt[:, 512:1024], in_=xh[half, d, :, 512:1024])
                        elif half == 0 and d < 4:
                            nc.sync.dma_start(out=xt[:, 0:512], in_=xh[half, d, :, 0:512])
                            nc.sync.dma_start(out=xt[:, 512:1024], in_=xh[half, d, :, 512:1024])
                        else:
                            nc.sync.dma_start(out=xt, in_=xh[half, d, :, :])
                        xh_t[(half, d)] = xt

                def rope_sq(p0, p1):
                    # squares on ACT, emitted right after the producing pass so
                    # they run while PE continues with the next matmul pass
                    sq0 = ropep.tile([128, 512], BF16, tag="sq0", bufs=3)
                    sq1 = ropep.tile([128, 512], BF16, tag="sq1", bufs=3)
                    nc.scalar.activation(sq0, p0, AF.Square)
                    nc.scalar.activation(sq1, p1, AF.Square)
                    return sq0, sq1

                def rope_stage(p0, p1, kind):
                    # stage PSUM -> SBUF bf16 on ACT with the rms (1+scale)
                    # fold-in: frees the projection banks long before the
                    # rotation chain runs, and makes the rotation pure
                    # tensor_tensor (fast bf16 path on DVE)
                    ki = 0 if kind == "q" else 1
                    pc0 = ropep.tile([128, 512], BF16, tag="pc0", bufs=4)
                    pc1 = ropep.tile([128, 512], BF16, tag="pc1", bufs=4)
                    nc.scalar.mul(pc0, p0, scs_sb[:, ki, 0:1])
                    nc.scalar.mul(pc1, p1, scs_sb[:, ki, 1:2])
                    return pc0, pc1

                def sq_add(sq01):
                    # DVE pre-add of the squared h-halves (fast bf16 path)
                    sqs = ropep.tile([128, 512], BF16, tag="sqs", bufs=3)
                    nc.vector.tensor_tensor(sqs, sq01[0], sq01[1], OP.add)
                    return sqs

                def rest_pss(sqs, kind, ptag="pss"):
                    # single ones-matmul partition-reduce + ACT rsqrt (bf16:
                    # keeps the sin/cos scaling on the fast all-bf16 DVE path)
                    pss = psum.tile([128, 512], F32, tag=ptag, bufs=1, name="pss")
                    nc.tensor.matmul(pss, ones_b, sqs, start=True, stop=True)
                    rs = ropep.tile([128, 512], BF16, tag="rs", bufs=3)
                    if kind == "q":
                        # 1/16 * rsqrt(ss/256 + eps) == 1/sqrt(ss + 256*eps)
                        nc.scalar.activation(rs, pss, AF.Abs_reciprocal_sqrt,
                                             scale=1.0, bias=bias_q)
                    else:
                        nc.scalar.activation(rs, pss, AF.Abs_reciprocal_sqrt,
                                             scale=1.0 / H, bias=bias_k)
                    return rs

                def rest_rot(pc01, rs, dst, qt):
                    # sin/cos scaling on Pool, all-bf16 tensor_tensor rotation
                    # on DVE; entirely off the projection critical path
                    pc0, pc1 = pc01
                    cs = ropep.tile([128, 512], BF16, tag="cs", bufs=4)
                    ss = ropep.tile([128, 512], BF16, tag="ss", bufs=4)
                    nc.vector.tensor_tensor(cs, cst_sb[qt][:, 0:512], rs, OP.mult)
                    nc.vector.tensor_tensor(ss, cst_sb[qt][:, 512:1024], rs, OP.mult)
                    t0 = ropep.tile([128, 512], BF16, tag="t0", bufs=4)
                    t1 = ropep.tile([128, 512], BF16, tag="t1", bufs=4)
                    nc.vector.tensor_tensor(t0, pc0, cs, OP.mult)
                    nc.vector.tensor_tensor(t1, pc1, ss, OP.mult)
                    nc.vector.tensor_tensor(dst[:, 0, :], t0, t1, OP.subtract)
                    t2 = ropep.tile([128, 512], BF16, tag="t0", bufs=4)
                    t3 = ropep.tile([128, 512], BF16, tag="t1", bufs=4)
                    nc.vector.tensor_tensor(t2, pc1, cs, OP.mult)
                    nc.vector.tensor_tensor(t3, pc0, ss, OP.mult)
                    nc.vector.tensor_tensor(dst[:, 1, :], t2, t3, OP.add)

                for qt in range(NQ):
                    tq = slice(512 * qt, 512 * (qt + 1))
                    half, qo = qt // 2, (qt % 2) * 512
                    xts = [xh_t[(half, d)][:, qo:qo + 512] for d in range(16)]
                    pk = [psum.tile([128, 512], F32, tag=f"pk{hh}", bufs=1,
                                    name=f"pk{qt}_{hh}") for hh in range(2)]
                    pq = [[psum.tile([128, 512], F32, tag=f"pq{nl}{hh}", bufs=1,
                                     name=f"pq{qt}_{nl}{hh}") for hh in range(2)]
                          for nl in range(2)]

                    def k_mm(d):
                        w = wA_t[d // 2][:, 0, d % 2]
                        st, sp = d == 0, d == 15
                        nc.tensor.matmul(pk[0], w[:, 0:128], xts[d],
                                         start=st, stop=sp)
                        nc.tensor.matmul(pk[1], w[:, 128:256], xts[d],
                                         start=st, stop=sp)

                    def q_mm(d):
                        st, sp = d == 0, d == 15
                        for nl in range(2):
                            w = wA_t[d // 2][:, 1 + nl, d % 2]
                            nc.tensor.matmul(pq[nl][0], w[:, 0:128], xts[d],
                                             start=st, stop=sp)
                            nc.tensor.matmul(pq[nl][1], w[:, 128:256], xts[d],
                                             start=st, stop=sp)

                    # v natural [t,h]: bf16 x-chunk stationary, vw moving;
                    # PSUM->SBUF copy on DVE (ACT stays squares+rsqrt only)
                    def v_half(hf, mid=None):
                        pvv = psum.tile([128, 2, H], F32, tag="pva", bufs=1,
                                        name=f"pvv{qt}_{hf}")
                        tc0 = 4 * qt + 2 * hf
                        for sub in range(2):
                            if sub == 1 and mid is not None:
                                mid()
                            tl = slice(128 * (2 * hf + sub), 128 * (2 * hf + sub) + 128)
                            for d in range(16):
                                nc.tensor.matmul(pvv[:, sub, :], xts[d][:, tl],
                                                 vw_sb[:, d, :],
                                                 start=(d == 0), stop=(d == 15))
                        nc.vector.tensor_copy(v_sb[:, tc0:tc0 + 2, :], pvv)

                    # quarter 0 interleaves k+q per d-chunk to pace with the x
                    # DMA stream; later quarters run the k pass first so its
                    # squares+staging free the pk banks during the q pass.
                    # Queue shapes per quarter:
                    #   ACT:  sqk2 pck2 | sq02 pc02 sq12 pc12 rs_k rs_q0 rs_q1
                    #   DVE:  v0cp rot_k v1cp rot_q0 rot_q1
                    #   PE :  k q | pss_k v0 pss_q0 v1 pss_q1
                    # so no in-order queue ever blocks a PE dependency.
                    kdst = kT_sb[:, :, tq]
                    if qt == 0:
                        for d in range(16):
                            k_mm(d)
                            q_mm(d)
                    else:
                        for d in range(16):
                            k_mm(d)
                    # last quarter: the q-head pss reductions ride the pk
                    # banks (free since their stage copies) so every rsqrt and
                    # the Exp table swap retire while v1 still streams, and
                    # phase B starts the moment the last matmul does
                    last = qt == NQ - 1
                    sqk = rope_sq(pk[0], pk[1])
                    pck = rope_stage(pk[0], pk[1], "k")
                    sqsk = sq_add(sqk)
                    if qt > 0:
                        for d in range(16):
                            q_mm(d)
                    sq0 = rope_sq(pq[0][0], pq[0][1])
                    pc0 = rope_stage(pq[0][0], pq[0][1], "q")
                    sqs0 = sq_add(sq0)
                    if last:
                        # last quarter: every pss reduction + rsqrt + the Exp
                        # table swap retires while v0/v1 still stream -> phase
                        # B opens with its exps (and pair-2's lp banks) live
                        sq1 = rope_sq(pq[1][0], pq[1][1])
                        pc1 = rope_stage(pq[1][0], pq[1][1], "q")
                        sqs1 = sq_add(sq1)
                        rs_k = rest_pss(sqsk, "k")
                        holder = []
                        v_half(0, mid=lambda: holder.append(
                            rest_pss(sqs0, "q", ptag="pk0")))
                        rs_q0 = holder[0]
                        rs_q1 = rest_pss(sqs1, "q", ptag="pk1")
                        nc.scalar.activation(dum, ones32[:, 0:1], AF.Exp)
                        v_half(1)
                    else:
                        rs_k = rest_pss(sqsk, "k")
                        v_half(0)
                        sq1 = rope_sq(pq[1][0], pq[1][1])
                        pc1 = rope_stage(pq[1][0], pq[1][1], "q")
                        sqs1 = sq_add(sq1)
                        rs_q0 = rest_pss(sqs0, "q")
                        v_half(1)
                        rs_q1 = rest_pss(sqs1, "q")
                    rest_rot(pck, rs_k, kdst, qt)
                    rest_rot(pc0, rs_q0, qT_sb[:, 0, :, tq], qt)
                    rest_rot(pc1, rs_q1, qT_sb[:, 1, :, tq], qt)

            # ---------------- Phases B + C ----------------
            # bulk weights needed only by phase B ride the SP ring behind the
            # x tiles, keeping the ACT ring (and its descriptor slots) small
            for nl in range(2):
                for hh in range(2):
                    nc.sync.dma_start(out=ow_sb[:, nl, hh, :], in_=ow[nl, hh, :, :])
            nc.sync.dma_start(out=masks_sb, in_=masks[:, :, :])

            with tc.tile_pool(name="persB", bufs=1) as persB, \
                 tc.tile_pool(name="expt", bufs=2) as expt, \
                 tc.tile_pool(name="bw", bufs=1) as bw, \
                 tc.tile_pool(name="oc", bufs=3) as oc:

                LP_TAGS = ["pk0", "pk1", "pq00", "pq01"]
                PV_TAGS = ["pq11", "pss", "pva"]
                lp_i = [0]
                pv_i = [0]

                def lp_tile(first=False):
                    if first:
                        # the opening pair cycles two banks only, so the next
                        # pair's logits start on banks no exp has touched
                        # (their release otherwise tick-rounds past the exps)
                        tag = LP_TAGS[lp_i[0] % 2]
                    else:
                        tag = LP_TAGS[lp_i[0] % 4]
                    t = psum.tile([128, 2, 256], F32, tag=tag,
                                  bufs=1, name=f"lp{lp_i[0]}")
                    lp_i[0] += 1
                    return t

                def pv_tile(shape, nm):
                    t = psum.tile(shape, F32, tag=PV_TAGS[pv_i[0] % 3],
                                  bufs=1, name=f"{nm}{pv_i[0]}")
                    pv_i[0] += 1
                    return t

                pvT_sb = persB.tile([128, 2, 2, T], BF16)
                Es, accs, etss = {}, {}, {}

                def emit_logits_exp(pi, first=False):
                    i = 2 * pi
                    q0 = 256 * pi
                    js, slot, nf = _slots(pi)
                    E = expt.tile([128, 2, 10, 256], BF16, tag="E", bufs=2,
                                  name=f"E{pi}")
                    ets = {}
                    for nl in range(2):
                        for k in range(0, len(js), 2):
                            jp = js[k:k + 2]
                            lp = lp_tile(first)
                            spans = [_span(i, j) for j in jp]
                            for x2, j in enumerate(jp):
                                lo, hi = spans[x2]
                                sj = slice(128 * j, 128 * (j + 1))
                                nc.tensor.matmul(lp[:, x2, lo:hi], kT_sb[:, 0, sj],
                                                 qT_sb[:, nl, 0, q0 + lo:q0 + hi],
                                                 start=True, stop=False)
                                nc.tensor.matmul(lp[:, x2, lo:hi], kT_sb[:, 1, sj],
                                                 qT_sb[:, nl, 1, q0 + lo:q0 + hi],
                                                 start=False, stop=True)
                            sl = [slot[j] for j in jp]
                            if spans == [(0, 256), (0, 256)] and sl[1] == sl[0] + 1:
                                nc.scalar.activation(E[:, nl, sl[0]:sl[0] + 2, :],
                                                     lp, AF.Exp)
                            else:
                                for x2, (lo, hi) in enumerate(spans):
                                    nc.scalar.activation(E[:, nl, sl[x2], lo:hi],
                                                         lp[:, x2, lo:hi], AF.Exp)
                            for x2, j in enumerate(jp):
                                lo, hi = spans[x2]
                                mi = _mask_idx(i, j)
                                ej = E[:, nl, sl[x2], lo:hi]
                                if mi is not None:
                                    m = masks_sb[:, mi, lo:hi]
                                    nc.vector.tensor_tensor(ej, ej, m, OP.mult)
                                ets[(nl, j)] = (ej, lo, hi)
                    Es[pi], etss[pi] = E, ets

                def emit_den_tree(pi):
                    # masked-exp sums over key blocks on DVE (bf16, SBUF-only)
                    _, _, nf = _slots(pi)
                    E = Es[pi]
                    acc = bw.tile([128, 2, 256], BF16, tag="acc", bufs=2,
                                  name=f"acc{pi}")
                    tt = nc.vector.tensor_tensor
                    if nf == 8:
                        a = bw.tile([128, 2, 4, 256], BF16, tag="dena", bufs=2)
                        b = bw.tile([128, 2, 2, 256], BF16, tag="denb", bufs=2)
                        tt(a, E[:, :, 0:4], E[:, :, 4:8], OP.add)
                        tt(b, a[:, :, 0:2], a[:, :, 2:4], OP.add)
                        tt(acc, b[:, :, 0], b[:, :, 1], OP.add)
                    elif nf == 7:
                        a = bw.tile([128, 2, 3, 256], BF16, tag="dena", bufs=2)
                        tt(a, E[:, :, 0:3], E[:, :, 3:6], OP.add)
                        tt(acc, a[:, :, 0], a[:, :, 1], OP.add)
                        tt(acc, acc, a[:, :, 2], OP.add)
                        tt(acc, acc, E[:, :, 6], OP.add)
                    elif nf == 5:
                        a = bw.tile([128, 2, 2, 256], BF16, tag="dena", bufs=2)
                        tt(a, E[:, :, 0:2], E[:, :, 2:4], OP.add)
                        tt(acc, a[:, :, 0], a[:, :, 1], OP.add)
                        tt(acc, acc, E[:, :, 4], OP.add)
                    elif nf == 3:
                        tt(acc, E[:, :, 0], E[:, :, 1], OP.add)
                        tt(acc, acc, E[:, :, 2], OP.add)
                    else:  # nf == 1 (pair 0)
                        nc.vector.tensor_copy(acc[:, :, 0:128], E[:, :, 0, 0:128])
                        tt(acc[:, :, 128:256], E[:, :, 0, 128:256],
                           E[:, :, 9, 128:256], OP.add)
                        accs[pi] = acc
                        return
                    i = 2 * pi
                    if i >= 8:
                        tt(acc[:, :, 0:128], acc[:, :, 0:128],
                           E[:, :, 8, 0:128], OP.add)
                    tt(acc[:, :, 128:256], acc[:, :, 128:256],
                       E[:, :, 9, 128:256], OP.add)
                    accs[pi] = acc

                def emit_tail(pi):
                    i = 2 * pi
                    tqs = slice(256 * pi, 256 * (pi + 1))
                    js = _jlist(i)
                    ets = etss[pi]
                    pd2 = psum.tile([128, 2, 256], F32, tag="pq10", bufs=1, name="pd2")
                    nc.tensor.matmul(pd2, ones_b, accs[pi], start=True, stop=True)
                    r2 = bw.tile([128, 2, 256], F32, tag="r2", bufs=2)
                    nc.vector.reciprocal_approx_fast(r2, pd2)
                    for nl in range(2):
                        for hh in range(2):
                            pv = pv_tile([128, 256], "pv")
                            hs = slice(128 * hh, 128 * (hh + 1))
                            for idx, j in enumerate(js):
                                ap, lo, hi = ets[(nl, j)]
                                nc.tensor.matmul(pv[:, lo:hi], v_sb[:, j, hs], ap,
                                                 start=(idx == 0),
                                                 stop=(idx == len(js) - 1),
                                                 skip_group_check=True)
                            nc.vector.tensor_tensor(pvT_sb[:, nl, hh, tqs], pv,
                                                    r2[:, nl, :], OP.mult)

                def emit_oproj(pi, fine=False):
                    # output projection for this pair's two token blocks;
                    # out-DMAs alternate the SP and ACT rings so the tail
                    # never serializes on one sequencer; the final block runs
                    # 256-col groups so its cast+DMA chain drains early and
                    # the kernel tail is one 64KB transfer
                    for tb in (2 * pi, 2 * pi + 1):
                        ts_ = slice(128 * tb, 128 * (tb + 1))
                        od = oc.tile([128, D], BF16, tag="od", bufs=3)
                        last_tb = fine and tb == 2 * pi + 1
                        nchunk, w = (8, 256) if last_tb else (4, 512)
                        for dt in range(nchunk):
                            dsl = slice(w * dt, w * (dt + 1))
                            po = pv_tile([128, w], "po")
                            step = 0
                            for nl in range(2):
                                for hh in range(2):
                                    nc.tensor.matmul(po, pvT_sb[:, nl, hh, ts_],
                                                     ow_sb[:, nl, hh, dsl],
                                                     start=(step == 0), stop=(step == 3))
                                    step += 1
                            nc.scalar.copy(od[:, dsl], po)
                            if dt % 2 == 0:
                                nc.sync.dma_start(out=out[tb, :, dsl],
                                                  in_=od[:, dsl])
                            else:
                                nc.scalar.dma_start(out=out[tb, :, dsl],
                                                    in_=od[:, dsl])

                # software pipeline over pairs [1..7, 0] (cheap pair lands
                # last): logits(p) fills PE while p-1's exps finish; the den
                # tree for p runs on DVE during p+1's logits; o-proj trails by
                # two pairs so its matmuls fill every exp/divide wait window.
                order = list(range(1, NPAIR)) + [0]
                for idx, pi in enumerate(order):
                    emit_logits_exp(pi, first=(idx == 0))
                    if idx == 0:
                        lp_i[0] = 2
                    if idx >= 1:
                        emit_tail(order[idx - 1])
                    emit_den_tree(pi)
                    if idx >= 2:
                        emit_oproj(order[idx - 2])
                emit_tail(order[-1])
                emit_oproj(order[-2])
                emit_oproj(order[-1], fine=True)

    nc.compile()
    return nc


_prog = None
last_results = None


def kernel(x, positions, q_w, k_w, v_w, o_w, q_norm_scale, k_norm_scale):
    global _prog, last_results
    x = np.asarray(x); positions = np.asarray(positions)
    q_w = np.asarray(q_w); k_w = np.asarray(k_w); v_w = np.asarray(v_w); o_w = np.asarray(o_w)
    q_norm_scale = np.asarray(q_norm_scale); k_norm_scale = np.asarray(k_norm_scale)

    if _prog is None:
        _prog = _build()
    nc = _prog

    # host-side constants
    j = np.arange(H // 2, dtype=np.float32)
    timescale = (BASE_FREQ ** (2.0 / H * j)).astype(np.float32)

    c = np.arange(128)[:, None]
    r = np.arange(128)[None, :]
    up = (c <= r).astype(np.float32)
    lo = (c > r).astype(np.float32)
    one_b = np.ones((128, 128), np.float32)
    zero_b = np.zeros((128, 128), np.float32)
    masks_np = np.stack([
        np.concatenate([lo, zero_b], 1),
        np.concatenate([one_b, lo], 1),
        np.concatenate([up, one_b], 1),
        np.concatenate([zero_b, up], 1),
    ], axis=0).transpose(1, 0, 2).astype(NPB)  # [128, 4, 256]

    scs_np = np.empty((128, 2, 2), np.float32)
    scs_np[:, 0, 0] = 1.0 + q_norm_scale[:128]
    scs_np[:, 0, 1] = 1.0 + q_norm_scale[128:]
    scs_np[:, 1, 0] = 1.0 + k_norm_scale[:128]
    scs_np[:, 1, 1] = 1.0 + k_norm_scale[128:]

    in_maps = []
    for core in range(8):
        b, tp = core // 4, core % 4
        sinu = positions[b].astype(np.float32)[:, None] / timescale[None, :]  # [T, 128]
        cos_np = np.cos(sinu).T.reshape(128, 4, 512).transpose(1, 0, 2)  # [4,128,512]
        sin_np = np.sin(sinu).T.reshape(128, 4, 512).transpose(1, 0, 2)
        cst_np = np.concatenate([cos_np, sin_np], axis=2).astype(NPB)  # [4,128,1024]
        # x^T packed [half, d, 128, 1024]
        xT = x[b].T.reshape(16, 128, 2, 1024).transpose(2, 0, 1, 3)
        # fused k/q weights, kind-major [c, 128, 3(kind), 2(dhalf), 256]
        wA_np = np.empty((8, 128, 3, 2, 256), np.float32)
        kw = k_w[tp]; qw = q_w[2 * tp:2 * tp + 2]
        wA_np[:, :, 0] = kw.reshape(8, 2, 128, 256).transpose(0, 2, 1, 3)
        wA_np[:, :, 1] = qw[0].reshape(8, 2, 128, 256).transpose(0, 2, 1, 3)
        wA_np[:, :, 2] = qw[1].reshape(8, 2, 128, 256).transpose(0, 2, 1, 3)
        ow_np = o_w[2 * tp:2 * tp + 2].reshape(2, 2, 128, D)  # [nl, hh, 128, D]
        in_maps.append({
            "xh": np.ascontiguousarray(xT).astype(NPB),
            "wA": np.ascontiguousarray(wA_np).astype(NPB),
            "vw": np.ascontiguousarray(
                v_w[tp].reshape(16, 128, 256).transpose(1, 0, 2)).astype(NPB),
            "ow": np.ascontiguousarray(ow_np).astype(NPB),
            "cst": np.ascontiguousarray(cst_np),
            "masks": masks_np,
            "scs": scs_np,
        })

    res = run_bass_kernel_spmd(nc, in_maps, core_ids=list(range(8)))
    last_results = res

    out = np.zeros((B, T, D), np.float32)
    for core in range(8):
        out[core // 4] += np.asarray(res.results[core]["out"]).astype(np.float32).reshape(T, D)
    return out


# revision 37
# speedup vs baseline: 1.0098x; 1.0098x over previous
"""Sliding-window GQA attention (B=2,T=2048,D=2048,N=8,K=4,H=256,W=1024) on 8 trn2 cores.

Sharding: batch over 2 (fsdp) x heads over 4 (tp). Core (b, tp) computes 2 q heads /
1 kv head for batch b; partial [T, D] outputs are summed over tp on the host.

All matmuls are bf16 x bf16 accumulating in f32 PSUM; bf16 ap-128 stays full-rate,
which lets phase B skip the guaranteed-masked halves of the two window-edge key
blocks. Inputs ship bf16, host-packed so every DMA moves contiguous >=1KB rows:
x halves on the SP ring (first tiles split for cold-queue parallelism), weights/
tables on the ACT ring in first-use order (k-parts and v-weight halves early).

PE-work minimization (vs the 277us revision):
  - softmax denominators: DVE pairwise-add tree over the masked exp tiles
    (all-bf16 SBUF tensor_tensor = fast DVE mode) + ONE ones-matmul per pair
    (512 rows) instead of per-key-block ones-matmuls (4608 rows/pair).
  - RMS sum-of-squares: DVE pre-adds the squared h-halves, single ones-matmul.
  - 1/den via nc.vector.reciprocal_approx_fast, so phase B's ACT runs Exp only:
    the Exp<->Abs_reciprocal_sqrt activation-table reloads (1.28us, 2/pair) are
    gone; both tables are pre-loaded via dummy ops off the critical path.
PE-idle minimization:
  - q/k PSUM banks are staged to SBUF bf16 on ACT right after the squares (rms
    (1+scale) folded into the copy); the RoPE rotation is pure bf16
    tensor_tensor on DVE reading staged copies, so projections never wait on
    the rotation chains.  The Pool engine is left IDLE on purpose: DVE and
    GpSimd share SBUF ports, and any Pool activity doubles DVE op latency.
  - one unified PSUM pool: phase B's tiles reuse phase A's bank tags, so B's
    first matmuls wait on one early-freed bank, not a pool-release barrier.
  - last quarter: q-head pss reductions ride the freed pk banks and the Exp
    table swap happens while v1 still streams, so B starts with the seam dry.
  - phase B runs software-pipelined: logits(p) | tail(p-1) | den-tree(p) |
    o-proj(p-2); o-proj casts on ACT; out-DMAs alternate SP/ACT rings, final
    block split per half so the kernel tail is one small transfer.
"""
import os

import numpy as np
import ml_dtypes

import concourse.bacc as bacc
import concourse.mybir as mybir
from concourse.tile import TileContext
from concourse.bass_utils import run_bass_kernel_spmd

try:  # pragma: no cover - profiling hook is optional
    from antenv.axon_hooks import get_axon_ntff_profile_hook  # noqa: F401
except ImportError:
    os.environ.setdefault("BASS_NEVER_TRACE", "1")


F32 = mybir.dt.float32
BF16 = mybir.dt.bfloat16
AF = mybir.ActivationFunctionType
OP = mybir.AluOpType

B, T, D = 2, 2048, 2048
N, KV, H = 8, 4, 256
WINDOW = 1024
BASE_FREQ = 10000.0
EPS = 1e-6
NB = T // 128          # 16 token blocks
NQ = 4                 # t quarters for projections (512 each)
NPAIR = 8              # query-block pairs (256 tokens each)
NPB = np.dtype(ml_dtypes.bfloat16)


def _mask_idx(i, j):
    if j == i + 1:
        return 3
    if j == i:
        return 2
    if j == i - 7:
        return 1
    if j == i - 8:
        return 0
    return None


def _jlist(i):
    return list(range(max(0, i - 8), i + 2))


def _span(i, j):
    # query half actually alive for this key block (128-granular)
    if j == i + 1:
        return 128, 256
    if j == i - 8:
        return 0, 128
    return 0, 256


def _slots(pi):
    # E-tile slot per key block: full-span blocks pack into slots 0..nf-1
    # (so the DVE add-tree reduces them with strided ops), lo-edge -> 8,
    # hi-edge -> 9.
    i = 2 * pi
    js = _jlist(i)
    slot, nf = {}, 0
    for j in js:
        lo, hi = _span(i, j)
        if (lo, hi) == (0, 256):
            slot[j] = nf
            nf += 1
        elif (lo, hi) == (0, 128):
            slot[j] = 8
        else:
            slot[j] = 9
    return js, slot, nf


def _build():
    nc = bacc.Bacc(None)

    xh = nc.dram_tensor("xh", [2, 16, 128, 1024], BF16, kind="ExternalInput")
    # fused k/q weights, kind-major: [c, 128, 3, 2, 256]; kind 0=kw, 1/2=qw
    wA = nc.dram_tensor("wA", [8, 128, 3, 2, 256], BF16, kind="ExternalInput")
    vw = nc.dram_tensor("vw", [128, 16, 256], BF16, kind="ExternalInput")
    ow = nc.dram_tensor("ow", [2, 2, 128, D], BF16, kind="ExternalInput")  # [nl,hh,128,D]
    cst = nc.dram_tensor("cst", [4, 128, 1024], BF16, kind="ExternalInput")
    masks = nc.dram_tensor("masks", [128, 4, 256], BF16, kind="ExternalInput")
    scs = nc.dram_tensor("scs", [128, 2, 2], F32, kind="ExternalInput")  # (1+scale)[q/k][hh]
    out = nc.dram_tensor("out", [16, 128, D], BF16, kind="ExternalOutput")

    with TileContext(nc) as tc:
        # one PSUM pool for the whole kernel: phase B reuses phase A's banks
        # by tag, so B's first matmuls wait only on the bank they reuse
        # (freed early) instead of a pool-release barrier over every psA user
        with tc.tile_pool(name="pers", bufs=1) as pers, \
             tc.tile_pool(name="ps", bufs=1, space="PSUM") as psum:
            kT_sb = pers.tile([128, 2, T], BF16)
            v_sb = pers.tile([128, NB, H], BF16)
            qT_sb = pers.tile([128, 2, 2, T], BF16)
            scs_sb = pers.tile([128, 2, 2], F32)
            ones32 = pers.tile([128, 128], F32)
            ones_b = pers.tile([128, 128], BF16)
            bias_q = pers.tile([128, 1], F32)
            bias_k = pers.tile([128, 1], F32)
            dum = pers.tile([128, 1], F32)
            masks_sb = pers.tile([128, 4, 256], BF16)
            ow_sb = pers.tile([128, 2, 2, D], BF16)

            nc.vector.memset(ones32, 1.0)
            nc.vector.tensor_copy(ones_b, ones32)
            nc.vector.memset(bias_q, float(H * EPS))
            nc.vector.memset(bias_k, EPS)

            # ---------------- Phase A: all projections + rms + rope ----------------
            with tc.tile_pool(name="wts", bufs=1) as wts, \
                 tc.tile_pool(name="xs", bufs=20) as xs, \
                 tc.tile_pool(name="ropep", bufs=1) as ropep:

                # ACT-ring DMAs in first-use order (x rides the SP ring).
                # k-parts of the fused weights ship first so k_mm(0) starts asap.
                wA_t = {}
                vw_sb = wts.tile([128, 16, 256], BF16)
                for c in range(8):
                    wA_t[c] = wts.tile([128, 3, 2, 256], BF16, tag=f"wA{c}", name=f"wA{c}")
                    if c == 0:
                        # finest first: cold DMA queues run at per-queue
                        # bandwidth, so the first matmul's weights ship alone
                        nc.scalar.dma_start(out=wA_t[c][:, 0, 0], in_=wA[c, :, 0, 0])
                        nc.scalar.dma_start(out=wA_t[c][:, 0, 1], in_=wA[c, :, 0, 1])
                        nc.scalar.dma_start(out=wA_t[c][:, 1], in_=wA[c, :, 1])
                        nc.scalar.dma_start(out=wA_t[c][:, 2], in_=wA[c, :, 2])
                    elif c == 1:
                        for kd in range(3):
                            nc.scalar.dma_start(out=wA_t[c][:, kd], in_=wA[c, :, kd])
                    elif c < 4:
                        nc.scalar.dma_start(out=wA_t[c][:, 0], in_=wA[c, :, 0])
                        nc.scalar.dma_start(out=wA_t[c][:, 1:3], in_=wA[c, :, 1:3])
                    else:
                        nc.scalar.dma_start(out=wA_t[c], in_=wA[c])
                    if c in (3, 5):
                        # v weights ride between the k/q tiles in two halves so
                        # they land well before quarter 0's v matmuls (a single
                        # 1MB transfer sits on one ~46GB/s queue for >20us)
                        h = 8 * (c == 5)
                        nc.scalar.dma_start(out=vw_sb[:, h:h + 8],
                                            in_=vw[:, h:h + 8])
                cst_sb = {}
                for qt in range(NQ):
                    cst_sb[qt] = ropep.tile([128, 1024], BF16, tag=f"cst{qt}", name=f"cst{qt}")
                    nc.scalar.dma_start(out=cst_sb[qt], in_=cst[qt, :, :])
                    if qt == 0:
                        nc.scalar.dma_start(out=scs_sb, in_=scs[:, :, :])
                        # pre-load the abs_rsqrt table once descriptors are out
                        nc.scalar.activation(dum, ones32[:, 0:1],
                                             AF.Abs_reciprocal_sqrt,
                                             scale=1.0, bias=bias_k)

                # x halves on the SP ring; the first four tiles ship as
                # 128KB halves so the cold (per-queue-bandwidth-limited) DMA
                # engines overlap and quarter 0 is never starved.
                xh_t = {}
                for half in range(2):
                    for d in range(16):
                        xt = xs.tile([128, 1024], BF16, tag="xt")
                        if half == 0 and d == 0:
                            nc.sync.dma_start(out=xt[:, 0:256], in_=xh[half, d, :, 0:256])
                            nc.sync.dma_start(out=xt[:, 256:512], in_=xh[half, d, :, 256:512])
                            nc.sync.dma_start(out=xt[:, 512:1024], in_=xh[half, d, :, 512:1024])
                        elif half == 0 and d < 4:
                            nc.sync.dma_start(out=xt[:, 0:512], in_=xh[half, d, :, 0:512])
                            nc.sync.dma_start(out=xt[:, 512:1024], in_=xh[half, d, :, 512:1024])
                        else:
                            nc.sync.dma_start(out=xt, in_=xh[half, d, :, :])
                        xh_t[(half, d)] = xt

                def rope_sq(p0, p1):
                    # squares on ACT, emitted right after the producing pass so
                    # they run while PE continues with the next matmul pass
                    sq0 = ropep.tile([128, 512], BF16, tag="sq0", bufs=3)
                    sq1 = ropep.tile([128, 512], BF16, tag="sq1", bufs=3)
                    nc.scalar.activation(sq0, p0, AF.Square)
                    nc.scalar.activation(sq1, p1, AF.Square)
                    return sq0, sq1

                def rope_stage(p0, p1, kind):
                    # stage PSUM -> SBUF bf16 on ACT with the rms (1+scale)
                    # fold-in: frees the projection banks long before the
                    # rotation chain runs, and makes the rotation pure
                    # tensor_tensor (fast bf16 path on DVE)
                    ki = 0 if kind == "q" else 1
                    pc0 = ropep.tile([128, 512], BF16, tag="pc0", bufs=4)
                    pc1 = ropep.tile([128, 512], BF16, tag="pc1", bufs=4)
                    nc.scalar.mul(pc0, p0, scs_sb[:, ki, 0:1])
                    nc.scalar.mul(pc1, p1, scs_sb[:, ki, 1:2])
                    return pc0, pc1

                def sq_add(sq01):
                    # DVE pre-add of the squared h-halves (fast bf16 path)
                    sqs = ropep.tile([128, 512], BF16, tag="sqs", bufs=3)
                    nc.vector.tensor_tensor(sqs, sq01[0], sq01[1], OP.add)
                    return sqs

                def rest_pss(sqs, kind, ptag="pss"):
                    # single ones-matmul partition-reduce + ACT rsqrt (bf16:
                    # keeps the sin/cos scaling on the fast all-bf16 DVE path)
                    pss = psum.tile([128, 512], F32, tag=ptag, bufs=1, name="pss")
                    nc.tensor.matmul(pss, ones_b, sqs, start=True, stop=True)
                    rs = ropep.tile([128, 512], BF16, tag="rs", bufs=3)
                    if kind == "q":
                        # 1/16 * rsqrt(ss/256 + eps) == 1/sqrt(ss + 256*eps)
                        nc.scalar.activation(rs, pss, AF.Abs_reciprocal_sqrt,
                                             scale=1.0, bias=bias_q)
                    else:
                        nc.scalar.activation(rs, pss, AF.Abs_reciprocal_sqrt,
                                             scale=1.0 / H, bias=bias_k)
                    return rs

                def rest_rot(pc01, rs, dst, qt):
                    # sin/cos scaling on Pool, all-bf16 tensor_tensor rotation
                    # on DVE; entirely off the projection critical path
                    pc0, pc1 = pc01
                    cs = ropep.tile([128, 512], BF16, tag="cs", bufs=4)
                    ss = ropep.tile([128, 512], BF16, tag="ss", bufs=4)
                    nc.vector.tensor_tensor(cs, cst_sb[qt][:, 0:512], rs, OP.mult)
                    nc.vector.tensor_tensor(ss, cst_sb[qt][:, 512:1024], rs, OP.mult)
                    t0 = ropep.tile([128, 512], BF16, tag="t0", bufs=4)
                    t1 = ropep.tile([128, 512], BF16, tag="t1", bufs=4)
                    nc.vector.tensor_tensor(t0, pc0, cs, OP.mult)
                    nc.vector.tensor_tensor(t1, pc1, ss, OP.mult)
                    nc.vector.tensor_tensor(dst[:, 0, :], t0, t1, OP.subtract)
                    t2 = ropep.tile([128, 512], BF16, tag="t0", bufs=4)
                    t3 = ropep.tile([128, 512], BF16, tag="t1", bufs=4)
                    nc.vector.tensor_tensor(t2, pc1, cs, OP.mult)
                    nc.vector.tensor_tensor(t3, pc0, ss, OP.mult)
                    nc.vector.tensor_tensor(dst[:, 1, :], t2, t3, OP.add)

                for qt in range(NQ):
                    tq = slice(512 * qt, 512 * (qt + 1))
                    half, qo = qt // 2, (qt % 2) * 512
                    xts = [xh_t[(half, d)][:, qo:qo + 512] for d in range(16)]
                    pk = [psum.tile([128, 512], F32, tag=f"pk{hh}", bufs=1,
                                    name=f"pk{qt}_{hh}") for hh in range(2)]
                    pq = [[psum.tile([128, 512], F32, tag=f"pq{nl}{hh}", bufs=1,
                                     name=f"pq{qt}_{nl}{hh}") for hh in range(2)]
                          for nl in range(2)]

                    def k_mm(d):
                        w = wA_t[d // 2][:, 0, d % 2]
                        st, sp = d == 0, d == 15
                        nc.tensor.matmul(pk[0], w[:, 0:128], xts[d],
                                         start=st, stop=sp)
                        nc.tensor.matmul(pk[1], w[:, 128:256], xts[d],
                                         start=st, stop=sp)

                    def q_mm(d):
                        st, sp = d == 0, d == 15
                        for nl in range(2):
                            w = wA_t[d // 2][:, 1 + nl, d % 2]
                            nc.tensor.matmul(pq[nl][0], w[:, 0:128], xts[d],
                                             start=st, stop=sp)
                            nc.tensor.matmul(pq[nl][1], w[:, 128:256], xts[d],
                                             start=st, stop=sp)

                    # v natural [t,h]: bf16 x-chunk stationary, vw moving;
                    # PSUM->SBUF copy on DVE (ACT stays squares+rsqrt only)
                    def v_half(hf, mid=None):
                        pvv = psum.tile([128, 2, H], F32, tag="pva", bufs=1,
                                        name=f"pvv{qt}_{hf}")
                        tc0 = 4 * qt + 2 * hf
                        for sub in range(2):
                            if sub == 1 and mid is not None:
                                mid()
                            tl = slice(128 * (2 * hf + sub), 128 * (2 * hf + sub) + 128)
                            for d in range(16):
                                nc.tensor.matmul(pvv[:, sub, :], xts[d][:, tl],
                                                 vw_sb[:, d, :],
                                                 start=(d == 0), stop=(d == 15))
                        nc.vector.tensor_copy(v_sb[:, tc0:tc0 + 2, :], pvv)

                    # quarter 0 interleaves k+q per d-chunk to pace with the x
                    # DMA stream; later quarters run the k pass first so its
                    # squares+staging free the pk banks during the q pass.
                    # Queue shapes per quarter:
                    #   ACT:  sqk2 pck2 | sq02 pc02 sq12 pc12 rs_k rs_q0 rs_q1
                    #   DVE:  v0cp rot_k v1cp rot_q0 rot_q1
                    #   PE :  k q | pss_k v0 pss_q0 v1 pss_q1
                    # so no in-order queue ever blocks a PE dependency.
                    kdst = kT_sb[:, :, tq]
                    if qt == 0:
                        for d in range(16):
                            k_mm(d)
                            q_mm(d)
                    else:
                        for d in range(16):
                            k_mm(d)
                    # last quarter: the q-head pss reductions ride the pk
                    # banks (free since their stage copies) so every rsqrt and
                    # the Exp table swap retire while v1 still streams, and
                    # phase B starts the moment the last matmul does
                    last = qt == NQ - 1
                    sqk = rope_sq(pk[0], pk[1])
                    pck = rope_stage(pk[0], pk[1], "k")
                    sqsk = sq_add(sqk)
                    if qt > 0:
                        for d in range(16):
                            q_mm(d)
                    sq0 = rope_sq(pq[0][0], pq[0][1])
                    pc0 = rope_stage(pq[0][0], pq[0][1], "q")
                    sqs0 = sq_add(sq0)
                    if last:
                        # last quarter: every pss reduction + rsqrt + the Exp
                        # table swap retires while v0/v1 still stream -> phase
                        # B opens with its exps (and pair-2's lp banks) live
                        sq1 = rope_sq(pq[1][0], pq[1][1])
                        pc1 = rope_stage(pq[1][0], pq[1][1], "q")
                        sqs1 = sq_add(sq1)
                        rs_k = rest_pss(sqsk, "k")
                        holder = []
                        v_half(0, mid=lambda: holder.append(
                            rest_pss(sqs0, "q", ptag="pk0")))
                        rs_q0 = holder[0]
                        rs_q1 = rest_pss(sqs1, "q", ptag="pk1")
                        nc.scalar.activation(dum, ones32[:, 0:1], AF.Exp)
                        v_half(1)
                    else:
                        rs_k = rest_pss(sqsk, "k")
                        v_half(0)
                        sq1 = rope_sq(pq[1][0], pq[1][1])
                        pc1 = rope_stage(pq[1][0], pq[1][1], "q")
                        sqs1 = sq_add(sq1)
                        rs_q0 = rest_pss(sqs0, "q")
                        v_half(1)
                        rs_q1 = rest_pss(sqs1, "q")
                    rest_rot(pck, rs_k, kdst, qt)
                    rest_rot(pc0, rs_q0, qT_sb[:, 0, :, tq], qt)
                    rest_rot(pc1, rs_q1, qT_sb[:, 1, :, tq], qt)

            # ---------------- Phases B + C ----------------
            # bulk weights needed only by phase B ride the SP ring behind the
            # x tiles, keeping the ACT ring (and its descriptor slots) small
            for nl in range(2):
                for hh in range(2):
                    nc.sync.dma_start(out=ow_sb[:, nl, hh, :], in_=ow[nl, hh, :, :])
            nc.sync.dma_start(out=masks_sb, in_=masks[:, :, :])

            with tc.tile_pool(name="persB", bufs=1) as persB, \
                 tc.tile_pool(name="expt", bufs=2) as expt, \
                 tc.tile_pool(name="bw", bufs=1) as bw, \
                 tc.tile_pool(name="oc", bufs=3) as oc:

                LP_TAGS = ["pk0", "pk1", "pq00", "pq01"]
                PV_TAGS = ["pq11", "pss", "pva"]
                lp_i = [0]
                pv_i = [0]

                def lp_tile(first=False):
                    if first:
                        # the opening pair cycles two banks only, so the next
                        # pair's logits start on banks no exp has touched
                        # (their release otherwise tick-rounds past the exps)
                        tag = LP_TAGS[lp_i[0] % 2]
                    else:
                        tag = LP_TAGS[lp_i[0] % 4]
                    t = psum.tile([128, 2, 256], F32, tag=tag,
                                  bufs=1, name=f"lp{lp_i[0]}")
                    lp_i[0] += 1
                    return t

                def pv_tile(shape, nm):
                    t = psum.tile(shape, F32, tag=PV_TAGS[pv_i[0] % 3],
                                  bufs=1, name=f"{nm}{pv_i[0]}")
                    pv_i[0] += 1
                    return t

                pvT_sb = persB.tile([128, 2, 2, T], BF16)
                Es, accs, etss = {}, {}, {}

                def emit_logits_exp(pi, first=False):
                    i = 2 * pi
                    q0 = 256 * pi
                    js, slot, nf = _slots(pi)
                    E = expt.tile([128, 2, 10, 256], BF16, tag="E", bufs=2,
                                  name=f"E{pi}")
                    ets = {}
                    for nl in range(2):
                        for k in range(0, len(js), 2):
                            jp = js[k:k + 2]
                            lp = lp_tile(first)
                            spans = [_span(i, j) for j in jp]
                            for x2, j in enumerate(jp):
                                lo, hi = spans[x2]
                                sj = slice(128 * j, 128 * (j + 1))
                                nc.tensor.matmul(lp[:, x2, lo:hi], kT_sb[:, 0, sj],
                                                 qT_sb[:, nl, 0, q0 + lo:q0 + hi],
                                                 start=True, stop=False)
                                nc.tensor.matmul(lp[:, x2, lo:hi], kT_sb[:, 1, sj],
                                                 qT_sb[:, nl, 1, q0 + lo:q0 + hi],
                                                 start=False, stop=True)
                            sl = [slot[j] for j in jp]
                            if spans == [(0, 256), (0, 256)] and sl[1] == sl[0] + 1:
                                nc.scalar.activation(E[:, nl, sl[0]:sl[0] + 2, :],
                                                     lp, AF.Exp)
                            else:
                                for x2, (lo, hi) in enumerate(spans):
                                    nc.scalar.activation(E[:, nl, sl[x2], lo:hi],
                                                         lp[:, x2, lo:hi], AF.Exp)
                            for x2, j in enumerate(jp):
                                lo, hi = spans[x2]
                                mi = _mask_idx(i, j)
                                ej = E[:, nl, sl[x2], lo:hi]
                                if mi is not None:
                                    m = masks_sb[:, mi, lo:hi]
                                    nc.vector.tensor_tensor(ej, ej, m, OP.mult)
                                ets[(nl, j)] = (ej, lo, hi)
                    Es[pi], etss[pi] = E, ets

                def emit_den_tree(pi):
                    # masked-exp sums over key blocks on DVE (bf16, SBUF-only)
                    _, _, nf = _slots(pi)
                    E = Es[pi]
                    acc = bw.tile([128, 2, 256], BF16, tag="acc", bufs=2,
                                  name=f"acc{pi}")
                    tt = nc.vector.tensor_tensor
                    if nf == 8:
                        a = bw.tile([128, 2, 4, 256], BF16, tag="dena", bufs=2)
                        b = bw.tile([128, 2, 2, 256], BF16, tag="denb", bufs=2)
                        tt(a, E[:, :, 0:4], E[:, :, 4:8], OP.add)
                        tt(b, a[:, :, 0:2], a[:, :, 2:4], OP.add)
                        tt(acc, b[:, :, 0], b[:, :, 1], OP.add)
                    elif nf == 7:
                        a = bw.tile([128, 2, 3, 256], BF16, tag="dena", bufs=2)
                        tt(a, E[:, :, 0:3], E[:, :, 3:6], OP.add)
                        tt(acc, a[:, :, 0], a[:, :, 1], OP.add)
                        tt(acc, acc, a[:, :, 2], OP.add)
                        tt(acc, acc, E[:, :, 6], OP.add)
                    elif nf == 5:
                        a = bw.tile([128, 2, 2, 256], BF16, tag="dena", bufs=2)
                        tt(a, E[:, :, 0:2], E[:, :, 2:4], OP.add)
                        tt(acc, a[:, :, 0], a[:, :, 1], OP.add)
                        tt(acc, acc, E[:, :, 4], OP.add)
                    elif nf == 3:
                        tt(acc, E[:, :, 0], E[:, :, 1], OP.add)
                        tt(acc, acc, E[:, :, 2], OP.add)
                    else:  # nf == 1 (pair 0)
                        nc.vector.tensor_copy(acc[:, :, 0:128], E[:, :, 0, 0:128])
                        tt(acc[:, :, 128:256], E[:, :, 0, 128:256],
                           E[:, :, 9, 128:256], OP.add)
                        accs[pi] = acc
                        return
                    i = 2 * pi
                    if i >= 8:
                        tt(acc[:, :, 0:128], acc[:, :, 0:128],
                           E[:, :, 8, 0:128], OP.add)
                    tt(acc[:, :, 128:256], acc[:, :, 128:256],
                       E[:, :, 9, 128:256], OP.add)
                    accs[pi] = acc

                def emit_tail(pi):
                    i = 2 * pi
                    tqs = slice(256 * pi, 256 * (pi + 1))
                    js = _jlist(i)
                    ets = etss[pi]
                    pd2 = psum.tile([128, 2, 256], F32, tag="pq10", bufs=1, name="pd2")
                    nc.tensor.matmul(pd2, ones_b, accs[pi], start=True, stop=True)
                    r2 = bw.tile([128, 2, 256], F32, tag="r2", bufs=2)
                    nc.vector.reciprocal_approx_fast(r2, pd2)
                    for nl in range(2):
                        for hh in range(2):
                            pv = pv_tile([128, 256], "pv")
                            hs = slice(128 * hh, 128 * (hh + 1))
                            for idx, j in enumerate(js):
                                ap, lo, hi = ets[(nl, j)]
                                nc.tensor.matmul(pv[:, lo:hi], v_sb[:, j, hs], ap,
                                                 start=(idx == 0),
                                                 stop=(idx == len(js) - 1),
                                                 skip_group_check=True)
                            nc.vector.tensor_tensor(pvT_sb[:, nl, hh, tqs], pv,
                                                    r2[:, nl, :], OP.mult)

                def emit_oproj(pi, fine=False):
                    # output projection for this pair's two token blocks;
                    # out-DMAs alternate the SP and ACT rings so the tail
                    # never serializes on one sequencer; the final block runs
                    # 256-col groups so its cast+DMA chain drains early and
                    # the kernel tail is one 64KB transfer
                    for tb in (2 * pi, 2 * pi + 1):
                        ts_ = slice(128 * tb, 128 * (tb + 1))
                        od = oc.tile([128, D], BF16, tag="od", bufs=3)
                        last_tb = fine and tb == 2 * pi + 1
                        nchunk, w = (8, 256) if last_tb else (4, 512)
                        for dt in range(nchunk):
                            dsl = slice(w * dt, w * (dt + 1))
                            po = pv_tile([128, w], "po")
                            step = 0
                            for nl in range(2):
                                for hh in range(2):
                                    nc.tensor.matmul(po, pvT_sb[:, nl, hh, ts_],
                                                     ow_sb[:, nl, hh, dsl],
                                                     start=(step == 0), stop=(step == 3))
                                    step += 1
                            nc.scalar.copy(od[:, dsl], po)
                            if dt % 2 == 0:
                                nc.sync.dma_start(out=out[tb, :, dsl],
                                                  in_=od[:, dsl])
                            else:
                                nc.scalar.dma_start(out=out[tb, :, dsl],
                                                    in_=od[:, dsl])

                # software pipeline over pairs [1..7, 0] (cheap pair lands
                # last): logits(p) fills PE while p-1's exps finish; the den
                # tree for p runs on DVE during p+1's logits; o-proj trails by
                # two pairs so its matmuls fill every exp/divide wait window.
                order = list(range(1, NPAIR)) + [0]
                for idx, pi in enumerate(order):
                    emit_logits_exp(pi, first=(idx == 0))
                    if idx == 0:
                        lp_i[0] = 2
                    if idx >= 1:
                        emit_tail(order[idx - 1])
                    emit_den_tree(pi)
                    if idx >= 2:
                        emit_oproj(order[idx - 2])
                emit_tail(order[-1])
                emit_oproj(order[-2])
                emit_oproj(order[-1], fine=True)

    nc.compile()
    return nc


_prog = None
last_results = None


def kernel(x, positions, q_w, k_w, v_w, o_w, q_norm_scale, k_norm_scale):
    global _prog, last_results
    x = np.asarray(x); positions = np.asarray(positions)
    q_w = np.asarray(q_w); k_w = np.asarray(k_w); v_w = np.asarray(v_w); o_w = np.asarray(o_w)
    q_norm_scale = np.asarray(q_norm_scale); k_norm_scale = np.asarray(k_norm_scale)

    if _prog is None:
        _prog = _build()
    nc = _prog

    # host-side constants
    j = np.arange(H // 2, dtype=np.float32)
    timescale = (BASE_FREQ ** (2.0 / H * j)).astype(np.float32)

    c = np.arange(128)[:, None]
    r = np.arange(128)[None, :]
    up = (c <= r).astype(np.float32)
    lo = (c > r).astype(np.float32)
    one_b = np.ones((128, 128), np.float32)
    zero_b = np.zeros((128, 128), np.float32)
    masks_np = np.stack([
        np.concatenate([lo, zero_b], 1),
        np.concatenate([one_b, lo], 1),
        np.concatenate([up, one_b], 1),
        np.concatenate([zero_b, up], 1),
    ], axis=0).transpose(1, 0, 2).astype(NPB)  # [128, 4, 256]

    scs_np = np.empty((128, 2, 2), np.float32)
    scs_np[:, 0, 0] = 1.0 + q_norm_scale[:128]
    scs_np[:, 0, 1] = 1.0 + q_norm_scale[128:]
    scs_np[:, 1, 0] = 1.0 + k_norm_scale[:128]
    scs_np[:, 1, 1] = 1.0 + k_norm_scale[128:]

    in_maps = []
    for core in range(8):
        b, tp = core // 4, core % 4
        sinu = positions[b].astype(np.float32)[:, None] / timescale[None, :]  # [T, 128]
        cos_np = np.cos(sinu).T.reshape(128, 4, 512).transpose(1, 0, 2)  # [4,128,512]
        sin_np = np.sin(sinu).T.reshape(128, 4, 512).transpose(1, 0, 2)
        cst_np = np.concatenate([cos_np, sin_np], axis=2).astype(NPB)  # [4,128,1024]
        # x^T packed [half, d, 128, 1024]
        xT = x[b].T.reshape(16, 128, 2, 1024).transpose(2, 0, 1, 3)
        # fused k/q weights, kind-major [c, 128, 3(kind), 2(dhalf), 256]
        wA_np = np.empty((8, 128, 3, 2, 256), np.float32)
        kw = k_w[tp]; qw = q_w[2 * tp:2 * tp + 2]
        wA_np[:, :, 0] = kw.reshape(8, 2, 128, 256).transpose(0, 2, 1, 3)
        wA_np[:, :, 1] = qw[0].reshape(8, 2, 128, 256).transpose(0, 2, 1, 3)
        wA_np[:, :, 2] = qw[1].reshape(8, 2, 128, 256).transpose(0, 2, 1, 3)
        ow_np = o_w[2 * tp:2 * tp + 2].reshape(2, 2, 128, D)  # [nl, hh, 128, D]
        in_maps.append({
            "xh": np.ascontiguousarray(xT).astype(NPB),
            "wA": np.ascontiguousarray(wA_np).astype(NPB),
            "vw": np.ascontiguousarray(
                v_w[tp].reshape(16, 128, 256).transpose(1, 0, 2)).astype(NPB),
            "ow": np.ascontiguousarray(ow_np).astype(NPB),
            "cst": np.ascontiguousarray(cst_np),
            "masks": masks_np,
            "scs": scs_np,
        })

    res = run_bass_kernel_spmd(nc, in_maps, core_ids=list(range(8)))
    last_results = res

    out = np.zeros((B, T, D), np.float32)
    for core in range(8):
        out[core // 4] += np.asarray(res.results[core]["out"]).astype(np.float32).reshape(T, D)
    return out


# revision 38
# speedup vs baseline: 1.0266x; 1.0166x over previous
"""Sliding-window GQA attention (B=2,T=2048,D=2048,N=8,K=4,H=256,W=1024) on 8 trn2 cores.

Sharding: batch over 2 (fsdp) x heads over 4 (tp). Core (b, tp) computes 2 q heads /
1 kv head for batch b; partial [T, D] outputs are summed over tp on the host.

All matmuls are bf16 x bf16 accumulating in f32 PSUM; bf16 ap-128 stays full-rate,
which lets phase B skip the guaranteed-masked halves of the two window-edge key
blocks. Inputs ship bf16, host-packed so every DMA moves contiguous >=1KB rows:
x halves on the SP ring (first tiles split for cold-queue parallelism), weights/
tables on the ACT ring in first-use order (k-parts and v-weight halves early).

PE-work minimization (vs the 277us revision):
  - softmax denominators: DVE pairwise-add tree over the masked exp tiles
    (all-bf16 SBUF tensor_tensor = fast DVE mode) + ONE ones-matmul per pair
    (512 rows) instead of per-key-block ones-matmuls (4608 rows/pair).
  - RMS sum-of-squares: DVE pre-adds the squared h-halves, single ones-matmul.
  - 1/den via nc.vector.reciprocal_approx_fast, so phase B's ACT runs Exp only:
    the Exp<->Abs_reciprocal_sqrt activation-table reloads (1.28us, 2/pair) are
    gone; both tables are pre-loaded via dummy ops off the critical path.
PE-idle minimization:
  - q/k PSUM banks are staged to SBUF bf16 on ACT right after the squares (rms
    (1+scale) folded into the copy); the RoPE rotation is pure bf16
    tensor_tensor on DVE reading staged copies, so projections never wait on
    the rotation chains.  The Pool engine is left IDLE on purpose: DVE and
    GpSimd share SBUF ports, and any Pool activity doubles DVE op latency.
  - one unified PSUM pool: phase B's tiles reuse phase A's bank tags, so B's
    first matmuls wait on one early-freed bank, not a pool-release barrier.
  - last quarter: q-head pss reductions ride the freed pk banks and the Exp
    table swap happens while v1 still streams, so B starts with the seam dry.
  - phase B runs software-pipelined: logits(p) | tail(p-1) | den-tree(p) |
    o-proj(p-2); o-proj casts on ACT; out-DMAs alternate SP/ACT rings, final
    block split per half so the kernel tail is one small transfer.
"""
import os

import numpy as np
import ml_dtypes

import concourse.bacc as bacc
import concourse.mybir as mybir
from concourse.tile import TileContext
from concourse.bass_utils import run_bass_kernel_spmd

try:  # pragma: no cover - profiling hook is optional
    from antenv.axon_hooks import get_axon_ntff_profile_hook  # noqa: F401
except ImportError:
    os.environ.setdefault("BASS_NEVER_TRACE", "1")


F32 = mybir.dt.float32
BF16 = mybir.dt.bfloat16
AF = mybir.ActivationFunctionType
OP = mybir.AluOpType

B, T, D = 2, 2048, 2048
N, KV, H = 8, 4, 256
WINDOW = 1024
BASE_FREQ = 10000.0
EPS = 1e-6
NB = T // 128          # 16 token blocks
NQ = 4                 # t quarters for projections (512 each)
NPAIR = 8              # query-block pairs (256 tokens each)
NPB = np.dtype(ml_dtypes.bfloat16)


def _mask_idx(i, j):
    if j == i + 1:
        return 3
    if j == i:
        return 2
    if j == i - 7:
        return 1
    if j == i - 8:
        return 0
    return None


def _jlist(i):
    return list(range(max(0, i - 8), i + 2))


def _span(i, j):
    # query half actually alive for this key block (128-granular)
    if j == i + 1:
        return 128, 256
    if j == i - 8:
        return 0, 128
    return 0, 256


def _slots(pi):
    # E-tile slot per key block: full-span blocks pack into slots 0..nf-1
    # (so the DVE add-tree reduces them with strided ops), lo-edge -> 8,
    # hi-edge -> 9.
    i = 2 * pi
    js = _jlist(i)
    slot, nf = {}, 0
    for j in js:
        lo, hi = _span(i, j)
        if (lo, hi) == (0, 256):
            slot[j] = nf
            nf += 1
        elif (lo, hi) == (0, 128):
            slot[j] = 8
        else:
            slot[j] = 9
    return js, slot, nf


def _build():
    nc = bacc.Bacc(None)

    xh = nc.dram_tensor("xh", [2, 16, 128, 1024], BF16, kind="ExternalInput")
    # fused k/q weights, kind-major: [c, 128, 3, 2, 256]; kind 0=kw, 1/2=qw
    wA = nc.dram_tensor("wA", [8, 128, 3, 2, 256], BF16, kind="ExternalInput")
    vw = nc.dram_tensor("vw", [128, 16, 256], BF16, kind="ExternalInput")
    ow = nc.dram_tensor("ow", [2, 2, 128, D], BF16, kind="ExternalInput")  # [nl,hh,128,D]
    cst = nc.dram_tensor("cst", [4, 128, 1024], BF16, kind="ExternalInput")
    masks = nc.dram_tensor("masks", [128, 4, 256], BF16, kind="ExternalInput")
    scs = nc.dram_tensor("scs", [128, 2, 2], F32, kind="ExternalInput")  # (1+scale)[q/k][hh]
    out = nc.dram_tensor("out", [16, 128, D], BF16, kind="ExternalOutput")

    with TileContext(nc) as tc:
        # one PSUM pool for the whole kernel: phase B reuses phase A's banks
        # by tag, so B's first matmuls wait only on the bank they reuse
        # (freed early) instead of a pool-release barrier over every psA user
        with tc.tile_pool(name="pers", bufs=1) as pers, \
             tc.tile_pool(name="ps", bufs=1, space="PSUM") as psum:
            kT_sb = pers.tile([128, 2, T], BF16)
            v_sb = pers.tile([128, NB, H], BF16)
            qT_sb = pers.tile([128, 2, 2, T], BF16)
            scs_sb = pers.tile([128, 2, 2], F32)
            ones32 = pers.tile([128, 128], F32)
            ones_b = pers.tile([128, 128], BF16)
            bias_q = pers.tile([128, 1], F32)
            bias_k = pers.tile([128, 1], F32)
            dum = pers.tile([128, 1], F32)
            masks_sb = pers.tile([128, 4, 256], BF16)
            ow_sb = pers.tile([128, 2, 2, D], BF16)

            nc.vector.memset(ones32, 1.0)
            nc.vector.tensor_copy(ones_b, ones32)
            nc.vector.memset(bias_q, float(H * EPS))
            nc.vector.memset(bias_k, EPS)

            # ---------------- Phase A: all projections + rms + rope ----------------
            with tc.tile_pool(name="wts", bufs=1) as wts, \
                 tc.tile_pool(name="xs", bufs=24) as xs, \
                 tc.tile_pool(name="ropep", bufs=1) as ropep:

                # ACT-ring DMAs in first-use order (x rides the SP ring).
                # k-parts of the fused weights ship first so k_mm(0) starts asap.
                wA_t = {}
                vw_sb = wts.tile([128, 16, 256], BF16)
                for c in range(8):
                    wA_t[c] = wts.tile([128, 3, 2, 256], BF16, tag=f"wA{c}", name=f"wA{c}")
                    if c == 0:
                        # finest first: cold DMA queues run at per-queue
                        # bandwidth, so the first matmul's weights ship alone
                        nc.scalar.dma_start(out=wA_t[c][:, 0, 0], in_=wA[c, :, 0, 0])
                        nc.scalar.dma_start(out=wA_t[c][:, 0, 1], in_=wA[c, :, 0, 1])
                        nc.scalar.dma_start(out=wA_t[c][:, 1], in_=wA[c, :, 1])
                        nc.scalar.dma_start(out=wA_t[c][:, 2], in_=wA[c, :, 2])
                    elif c == 1:
                        for kd in range(3):
                            nc.scalar.dma_start(out=wA_t[c][:, kd], in_=wA[c, :, kd])
                    elif c < 4:
                        nc.scalar.dma_start(out=wA_t[c][:, 0], in_=wA[c, :, 0])
                        nc.scalar.dma_start(out=wA_t[c][:, 1:3], in_=wA[c, :, 1:3])
                    else:
                        nc.scalar.dma_start(out=wA_t[c], in_=wA[c])
                    if c in (3, 5):
                        # v weights ride between the k/q tiles in two halves so
                        # they land well before quarter 0's v matmuls (a single
                        # 1MB transfer sits on one ~46GB/s queue for >20us)
                        h = 8 * (c == 5)
                        nc.scalar.dma_start(out=vw_sb[:, h:h + 8],
                                            in_=vw[:, h:h + 8])
                cst_sb = {}
                for qt in range(NQ):
                    cst_sb[qt] = ropep.tile([128, 1024], BF16, tag=f"cst{qt}", name=f"cst{qt}")
                    nc.scalar.dma_start(out=cst_sb[qt], in_=cst[qt, :, :])
                    if qt == 0:
                        nc.scalar.dma_start(out=scs_sb, in_=scs[:, :, :])
                        # pre-load the abs_rsqrt table once descriptors are out
                        nc.scalar.activation(dum, ones32[:, 0:1],
                                             AF.Abs_reciprocal_sqrt,
                                             scale=1.0, bias=bias_k)

                # x halves on the SP ring; the first four tiles ship as
                # 128KB halves so the cold (per-queue-bandwidth-limited) DMA
                # engines overlap and quarter 0 is never starved.
                xh_t = {}
                for half in range(2):
                    for d in range(16):
                        xt = xs.tile([128, 1024], BF16, tag="xt")
                        if half == 0 and d == 0:
                            nc.sync.dma_start(out=xt[:, 0:256], in_=xh[half, d, :, 0:256])
                            nc.sync.dma_start(out=xt[:, 256:512], in_=xh[half, d, :, 256:512])
                            nc.sync.dma_start(out=xt[:, 512:1024], in_=xh[half, d, :, 512:1024])
                        elif half == 0 and d < 4:
                            nc.sync.dma_start(out=xt[:, 0:512], in_=xh[half, d, :, 0:512])
                            nc.sync.dma_start(out=xt[:, 512:1024], in_=xh[half, d, :, 512:1024])
                        else:
                            nc.sync.dma_start(out=xt, in_=xh[half, d, :, :])
                        xh_t[(half, d)] = xt

                def rope_sq(p0, p1):
                    # squares on ACT, emitted right after the producing pass so
                    # they run while PE continues with the next matmul pass
                    sq0 = ropep.tile([128, 512], BF16, tag="sq0", bufs=3)
                    sq1 = ropep.tile([128, 512], BF16, tag="sq1", bufs=3)
                    nc.scalar.activation(sq0, p0, AF.Square)
                    nc.scalar.activation(sq1, p1, AF.Square)
                    return sq0, sq1

                def rope_stage(p0, p1, kind):
                    # stage PSUM -> SBUF bf16 on ACT with the rms (1+scale)
                    # fold-in: frees the projection banks long before the
                    # rotation chain runs, and makes the rotation pure
                    # tensor_tensor (fast bf16 path on DVE)
                    ki = 0 if kind == "q" else 1
                    pc0 = ropep.tile([128, 512], BF16, tag="pc0", bufs=4)
                    pc1 = ropep.tile([128, 512], BF16, tag="pc1", bufs=4)
                    nc.scalar.mul(pc0, p0, scs_sb[:, ki, 0:1])
                    nc.scalar.mul(pc1, p1, scs_sb[:, ki, 1:2])
                    return pc0, pc1

                def sq_add(sq01):
                    # DVE pre-add of the squared h-halves (fast bf16 path)
                    sqs = ropep.tile([128, 512], BF16, tag="sqs", bufs=3)
                    nc.vector.tensor_tensor(sqs, sq01[0], sq01[1], OP.add)
                    return sqs

                def rest_pss(sqs, kind, ptag="pss"):
                    # single ones-matmul partition-reduce + ACT rsqrt (bf16:
                    # keeps the sin/cos scaling on the fast all-bf16 DVE path)
                    pss = psum.tile([128, 512], F32, tag=ptag, bufs=1, name="pss")
                    nc.tensor.matmul(pss, ones_b, sqs, start=True, stop=True)
                    rs = ropep.tile([128, 512], BF16, tag="rs", bufs=3)
                    if kind == "q":
                        # 1/16 * rsqrt(ss/256 + eps) == 1/sqrt(ss + 256*eps)
                        nc.scalar.activation(rs, pss, AF.Abs_reciprocal_sqrt,
                                             scale=1.0, bias=bias_q)
                    else:
                        nc.scalar.activation(rs, pss, AF.Abs_reciprocal_sqrt,
                                             scale=1.0 / H, bias=bias_k)
                    return rs

                def rest_rot(pc01, rs, dst, qt):
                    # sin/cos scaling on Pool, all-bf16 tensor_tensor rotation
                    # on DVE; entirely off the projection critical path
                    pc0, pc1 = pc01
                    cs = ropep.tile([128, 512], BF16, tag="cs", bufs=4)
                    ss = ropep.tile([128, 512], BF16, tag="ss", bufs=4)
                    nc.vector.tensor_tensor(cs, cst_sb[qt][:, 0:512], rs, OP.mult)
                    nc.vector.tensor_tensor(ss, cst_sb[qt][:, 512:1024], rs, OP.mult)
                    t0 = ropep.tile([128, 512], BF16, tag="t0", bufs=4)
                    t1 = ropep.tile([128, 512], BF16, tag="t1", bufs=4)
                    nc.vector.tensor_tensor(t0, pc0, cs, OP.mult)
                    nc.vector.tensor_tensor(t1, pc1, ss, OP.mult)
                    nc.vector.tensor_tensor(dst[:, 0, :], t0, t1, OP.subtract)
                    t2 = ropep.tile([128, 512], BF16, tag="t0", bufs=4)
                    t3 = ropep.tile([128, 512], BF16, tag="t1", bufs=4)
                    nc.vector.tensor_tensor(t2, pc1, cs, OP.mult)
                    nc.vector.tensor_tensor(t3, pc0, ss, OP.mult)
                    nc.vector.tensor_tensor(dst[:, 1, :], t2, t3, OP.add)

                for qt in range(NQ):
                    tq = slice(512 * qt, 512 * (qt + 1))
                    half, qo = qt // 2, (qt % 2) * 512
                    xts = [xh_t[(half, d)][:, qo:qo + 512] for d in range(16)]
                    pk = [psum.tile([128, 512], F32, tag=f"pk{hh}", bufs=1,
                                    name=f"pk{qt}_{hh}") for hh in range(2)]
                    pq = [[psum.tile([128, 512], F32, tag=f"pq{nl}{hh}", bufs=1,
                                     name=f"pq{qt}_{nl}{hh}") for hh in range(2)]
                          for nl in range(2)]

                    def k_mm(d):
                        w = wA_t[d // 2][:, 0, d % 2]
                        st, sp = d == 0, d == 15
                        nc.tensor.matmul(pk[0], w[:, 0:128], xts[d],
                                         start=st, stop=sp)
                        nc.tensor.matmul(pk[1], w[:, 128:256], xts[d],
                                         start=st, stop=sp)

                    def q_mm(d):
                        st, sp = d == 0, d == 15
                        for nl in range(2):
                            w = wA_t[d // 2][:, 1 + nl, d % 2]
                            nc.tensor.matmul(pq[nl][0], w[:, 0:128], xts[d],
                                             start=st, stop=sp)
                            nc.tensor.matmul(pq[nl][1], w[:, 128:256], xts[d],
                                             start=st, stop=sp)

                    # v natural [t,h]: bf16 x-chunk stationary, vw moving;
                    # PSUM->SBUF copy on DVE (ACT stays squares+rsqrt only)
                    def v_half(hf, mid=None):
                        pvv = psum.tile([128, 2, H], F32, tag="pva", bufs=1,
                                        name=f"pvv{qt}_{hf}")
                        tc0 = 4 * qt + 2 * hf
                        for sub in range(2):
                            if sub == 1 and mid is not None:
                                mid()
                            tl = slice(128 * (2 * hf + sub), 128 * (2 * hf + sub) + 128)
                            for d in range(16):
                                nc.tensor.matmul(pvv[:, sub, :], xts[d][:, tl],
                                                 vw_sb[:, d, :],
                                                 start=(d == 0), stop=(d == 15))
                        nc.vector.tensor_copy(v_sb[:, tc0:tc0 + 2, :], pvv)

                    # quarter 0 interleaves k+q per d-chunk to pace with the x
                    # DMA stream; later quarters run the k pass first so its
                    # squares+staging free the pk banks during the q pass.
                    # Queue shapes per quarter:
                    #   ACT:  sqk2 pck2 | sq02 pc02 sq12 pc12 rs_k rs_q0 rs_q1
                    #   DVE:  v0cp rot_k v1cp rot_q0 rot_q1
                    #   PE :  k q | pss_k v0 pss_q0 v1 pss_q1
                    # so no in-order queue ever blocks a PE dependency.
                    kdst = kT_sb[:, :, tq]
                    if qt == 0:
                        for d in range(16):
                            k_mm(d)
                            q_mm(d)
                    else:
                        for d in range(16):
                            k_mm(d)
                    # last quarter: the q-head pss reductions ride the pk
                    # banks (free since their stage copies) so every rsqrt and
                    # the Exp table swap retire while v1 still streams, and
                    # phase B starts the moment the last matmul does
                    last = qt == NQ - 1
                    sqk = rope_sq(pk[0], pk[1])
                    pck = rope_stage(pk[0], pk[1], "k")
                    sqsk = sq_add(sqk)
                    if qt > 0:
                        for d in range(16):
                            q_mm(d)
                    sq0 = rope_sq(pq[0][0], pq[0][1])
                    pc0 = rope_stage(pq[0][0], pq[0][1], "q")
                    sqs0 = sq_add(sq0)
                    if last:
                        # last quarter: every pss reduction + rsqrt + the Exp
                        # table swap retires while v0/v1 still stream -> phase
                        # B opens with its exps (and pair-2's lp banks) live
                        sq1 = rope_sq(pq[1][0], pq[1][1])
                        pc1 = rope_stage(pq[1][0], pq[1][1], "q")
                        sqs1 = sq_add(sq1)
                        rs_k = rest_pss(sqsk, "k")
                        holder = []
                        v_half(0, mid=lambda: holder.append(
                            rest_pss(sqs0, "q", ptag="pk0")))
                        rs_q0 = holder[0]
                        rs_q1 = rest_pss(sqs1, "q", ptag="pk1")
                        nc.scalar.activation(dum, ones32[:, 0:1], AF.Exp)
                        v_half(1)
                    else:
                        rs_k = rest_pss(sqsk, "k")
                        v_half(0)
                        sq1 = rope_sq(pq[1][0], pq[1][1])
                        pc1 = rope_stage(pq[1][0], pq[1][1], "q")
                        sqs1 = sq_add(sq1)
                        rs_q0 = rest_pss(sqs0, "q")
                        v_half(1)
                        rs_q1 = rest_pss(sqs1, "q")
                    rest_rot(pck, rs_k, kdst, qt)
                    rest_rot(pc0, rs_q0, qT_sb[:, 0, :, tq], qt)
                    rest_rot(pc1, rs_q1, qT_sb[:, 1, :, tq], qt)

            # ---------------- Phases B + C ----------------
            # bulk weights needed only by phase B ride the SP ring behind the
            # x tiles, keeping the ACT ring (and its descriptor slots) small
            for nl in range(2):
                for hh in range(2):
                    nc.sync.dma_start(out=ow_sb[:, nl, hh, :], in_=ow[nl, hh, :, :])
            nc.sync.dma_start(out=masks_sb, in_=masks[:, :, :])

            with tc.tile_pool(name="persB", bufs=1) as persB, \
                 tc.tile_pool(name="expt", bufs=2) as expt, \
                 tc.tile_pool(name="bw", bufs=1) as bw, \
                 tc.tile_pool(name="oc", bufs=3) as oc:

                LP_TAGS = ["pk0", "pk1", "pq00", "pq01"]
                PV_TAGS = ["pq11", "pss", "pva"]
                lp_i = [0]
                pv_i = [0]

                def lp_tile(first=False):
                    if first:
                        # the opening pair cycles two banks only, so the next
                        # pair's logits start on banks no exp has touched
                        # (their release otherwise tick-rounds past the exps)
                        tag = LP_TAGS[lp_i[0] % 2]
                    else:
                        tag = LP_TAGS[lp_i[0] % 4]
                    t = psum.tile([128, 2, 256], F32, tag=tag,
                                  bufs=1, name=f"lp{lp_i[0]}")
                    lp_i[0] += 1
                    return t

                def pv_tile(shape, nm):
                    t = psum.tile(shape, F32, tag=PV_TAGS[pv_i[0] % 3],
                                  bufs=1, name=f"{nm}{pv_i[0]}")
                    pv_i[0] += 1
                    return t

                pvT_sb = persB.tile([128, 2, 2, T], BF16)
                Es, accs, etss = {}, {}, {}

                def emit_logits_exp(pi, first=False):
                    i = 2 * pi
                    q0 = 256 * pi
                    js, slot, nf = _slots(pi)
                    E = expt.tile([128, 2, 10, 256], BF16, tag="E", bufs=2,
                                  name=f"E{pi}")
                    ets = {}
                    for nl in range(2):
                        for k in range(0, len(js), 2):
                            jp = js[k:k + 2]
                            lp = lp_tile(first)
                            spans = [_span(i, j) for j in jp]
                            for x2, j in enumerate(jp):
                                lo, hi = spans[x2]
                                sj = slice(128 * j, 128 * (j + 1))
                                nc.tensor.matmul(lp[:, x2, lo:hi], kT_sb[:, 0, sj],
                                                 qT_sb[:, nl, 0, q0 + lo:q0 + hi],
                                                 start=True, stop=False)
                                nc.tensor.matmul(lp[:, x2, lo:hi], kT_sb[:, 1, sj],
                                                 qT_sb[:, nl, 1, q0 + lo:q0 + hi],
                                                 start=False, stop=True)
                            sl = [slot[j] for j in jp]
                            if spans == [(0, 256), (0, 256)] and sl[1] == sl[0] + 1:
                                nc.scalar.activation(E[:, nl, sl[0]:sl[0] + 2, :],
                                                     lp, AF.Exp)
                            else:
                                for x2, (lo, hi) in enumerate(spans):
                                    nc.scalar.activation(E[:, nl, sl[x2], lo:hi],
                                                         lp[:, x2, lo:hi], AF.Exp)
                            for x2, j in enumerate(jp):
                                lo, hi = spans[x2]
                                mi = _mask_idx(i, j)
                                ej = E[:, nl, sl[x2], lo:hi]
                                if mi is not None:
                                    m = masks_sb[:, mi, lo:hi]
                                    nc.vector.tensor_tensor(ej, ej, m, OP.mult)
                                ets[(nl, j)] = (ej, lo, hi)
                    Es[pi], etss[pi] = E, ets

                def emit_den_tree(pi):
                    # masked-exp sums over key blocks on DVE (bf16, SBUF-only)
                    _, _, nf = _slots(pi)
                    E = Es[pi]
                    acc = bw.tile([128, 2, 256], BF16, tag="acc", bufs=2,
                                  name=f"acc{pi}")
                    tt = nc.vector.tensor_tensor
                    if nf == 8:
                        a = bw.tile([128, 2, 4, 256], BF16, tag="dena", bufs=2)
                        b = bw.tile([128, 2, 2, 256], BF16, tag="denb", bufs=2)
                        tt(a, E[:, :, 0:4], E[:, :, 4:8], OP.add)
                        tt(b, a[:, :, 0:2], a[:, :, 2:4], OP.add)
                        tt(acc, b[:, :, 0], b[:, :, 1], OP.add)
                    elif nf == 7:
                        a = bw.tile([128, 2, 3, 256], BF16, tag="dena", bufs=2)
                        tt(a, E[:, :, 0:3], E[:, :, 3:6], OP.add)
                        tt(acc, a[:, :, 0], a[:, :, 1], OP.add)
                        tt(acc, acc, a[:, :, 2], OP.add)
                        tt(acc, acc, E[:, :, 6], OP.add)
                    elif nf == 5:
                        a = bw.tile([128, 2, 2, 256], BF16, tag="dena", bufs=2)
                        tt(a, E[:, :, 0:2], E[:, :, 2:4], OP.add)
                        tt(acc, a[:, :, 0], a[:, :, 1], OP.add)
                        tt(acc, acc, E[:, :, 4], OP.add)
                    elif nf == 3:
                        tt(acc, E[:, :, 0], E[:, :, 1], OP.add)
                        tt(acc, acc, E[:, :, 2], OP.add)
                    else:  # nf == 1 (pair 0)
                        nc.vector.tensor_copy(acc[:, :, 0:128], E[:, :, 0, 0:128])
                        tt(acc[:, :, 128:256], E[:, :, 0, 128:256],
                           E[:, :, 9, 128:256], OP.add)
                        accs[pi] = acc
                        return
                    i = 2 * pi
                    if i >= 8:
                        tt(acc[:, :, 0:128], acc[:, :, 0:128],
                           E[:, :, 8, 0:128], OP.add)
                    tt(acc[:, :, 128:256], acc[:, :, 128:256],
                       E[:, :, 9, 128:256], OP.add)
                    accs[pi] = acc

                def emit_tail(pi):
                    i = 2 * pi
                    tqs = slice(256 * pi, 256 * (pi + 1))
                    js = _jlist(i)
                    ets = etss[pi]
                    pd2 = psum.tile([128, 2, 256], F32, tag="pq10", bufs=1, name="pd2")
                    nc.tensor.matmul(pd2, ones_b, accs[pi], start=True, stop=True)
                    r2 = bw.tile([128, 2, 256], F32, tag="r2", bufs=2)
                    nc.vector.reciprocal_approx_fast(r2, pd2)
                    for nl in range(2):
                        for hh in range(2):
                            pv = pv_tile([128, 256], "pv")
                            hs = slice(128 * hh, 128 * (hh + 1))
                            for idx, j in enumerate(js):
                                ap, lo, hi = ets[(nl, j)]
                                nc.tensor.matmul(pv[:, lo:hi], v_sb[:, j, hs], ap,
                                                 start=(idx == 0),
                                                 stop=(idx == len(js) - 1),
                                                 skip_group_check=True)
                            nc.vector.tensor_tensor(pvT_sb[:, nl, hh, tqs], pv,
                                                    r2[:, nl, :], OP.mult)

                def emit_oproj(pi, fine=False):
                    # output projection for this pair's two token blocks;
                    # out-DMAs alternate the SP and ACT rings so the tail
                    # never serializes on one sequencer; the final block runs
                    # 256-col groups so its cast+DMA chain drains early and
                    # the kernel tail is one 64KB transfer
                    for tb in (2 * pi, 2 * pi + 1):
                        ts_ = slice(128 * tb, 128 * (tb + 1))
                        od = oc.tile([128, D], BF16, tag="od", bufs=3)
                        last_tb = fine and tb == 2 * pi + 1
                        nchunk, w = (8, 256) if last_tb else (4, 512)
                        for dt in range(nchunk):
                            dsl = slice(w * dt, w * (dt + 1))
                            po = pv_tile([128, w], "po")
                            step = 0
                            for nl in range(2):
                                for hh in range(2):
                                    nc.tensor.matmul(po, pvT_sb[:, nl, hh, ts_],
                                                     ow_sb[:, nl, hh, dsl],
                                                     start=(step == 0), stop=(step == 3))
                                    step += 1
                            nc.scalar.copy(od[:, dsl], po)
                            if dt % 2 == 0:
                                nc.sync.dma_start(out=out[tb, :, dsl],
                                                  in_=od[:, dsl])
                            else:
                                nc.scalar.dma_start(out=out[tb, :, dsl],
                                                    in_=od[:, dsl])

                # software pipeline over pairs [1..7, 0] (cheap pair lands
                # last): logits(p) fills PE while p-1's exps finish; the den
                # tree for p runs on DVE during p+1's logits; o-proj trails by
                # two pairs so its matmuls fill every exp/divide wait window.
                order = list(range(1, NPAIR)) + [0]
                for idx, pi in enumerate(order):
                    emit_logits_exp(pi, first=(idx == 0))
                    if idx == 0:
                        lp_i[0] = 2
                    if idx >= 1:
                        emit_tail(order[idx - 1])
                    emit_den_tree(pi)
                    if idx >= 2:
                        emit_oproj(order[idx - 2])
                emit_tail(order[-1])
                emit_oproj(order[-2])
                emit_oproj(order[-1], fine=True)

    nc.compile()
    return nc


_prog = None
last_results = None


def kernel(x, positions, q_w, k_w, v_w, o_w, q_norm_scale, k_norm_scale):
    global _prog, last_results
    x = np.asarray(x); positions = np.asarray(positions)
    q_w = np.asarray(q_w); k_w = np.asarray(k_w); v_w = np.asarray(v_w); o_w = np.asarray(o_w)
    q_norm_scale = np.asarray(q_norm_scale); k_norm_scale = np.asarray(k_norm_scale)

    if _prog is None:
        _prog = _build()
    nc = _prog

    # host-side constants
    j = np.arange(H // 2, dtype=np.float32)
    timescale = (BASE_FREQ ** (2.0 / H * j)).astype(np.float32)

    c = np.arange(128)[:, None]
    r = np.arange(128)[None, :]
    up = (c <= r).astype(np.float32)
    lo = (c > r).astype(np.float32)
    one_b = np.ones((128, 128), np.float32)
    zero_b = np.zeros((128, 128), np.float32)
    masks_np = np.stack([
        np.concatenate([lo, zero_b], 1),
        np.concatenate([one_b, lo], 1),
        np.concatenate([up, one_b], 1),
        np.concatenate([zero_b, up], 1),
    ], axis=0).transpose(1, 0, 2).astype(NPB)  # [128, 4, 256]

    scs_np = np.empty((128, 2, 2), np.float32)
    scs_np[:, 0, 0] = 1.0 + q_norm_scale[:128]
    scs_np[:, 0, 1] = 1.0 + q_norm_scale[128:]
    scs_np[:, 1, 0] = 1.0 + k_norm_scale[:128]
    scs_np[:, 1, 1] = 1.0 + k_norm_scale[128:]

    in_maps = []
    for core in range(8):
        b, tp = core // 4, core % 4
        sinu = positions[b].astype(np.float32)[:, None] / timescale[None, :]  # [T, 128]
        cos_np = np.cos(sinu).T.reshape(128, 4, 512).transpose(1, 0, 2)  # [4,128,512]
        sin_np = np.sin(sinu).T.reshape(128, 4, 512).transpose(1, 0, 2)
        cst_np = np.concatenate([cos_np, sin_np], axis=2).astype(NPB)  # [4,128,1024]
        # x^T packed [half, d, 128, 1024]
        xT = x[b].T.reshape(16, 128, 2, 1024).transpose(2, 0, 1, 3)
        # fused k/q weights, kind-major [c, 128, 3(kind), 2(dhalf), 256]
        wA_np = np.empty((8, 128, 3, 2, 256), np.float32)
        kw = k_w[tp]; qw = q_w[2 * tp:2 * tp + 2]
        wA_np[:, :, 0] = kw.reshape(8, 2, 128, 256).transpose(0, 2, 1, 3)
        wA_np[:, :, 1] = qw[0].reshape(8, 2, 128, 256).transpose(0, 2, 1, 3)
        wA_np[:, :, 2] = qw[1].reshape(8, 2, 128, 256).transpose(0, 2, 1, 3)
        ow_np = o_w[2 * tp:2 * tp + 2].reshape(2, 2, 128, D)  # [nl, hh, 128, D]
        in_maps.append({
            "xh": np.ascontiguousarray(xT).astype(NPB),
            "wA": np.ascontiguousarray(wA_np).astype(NPB),
            "vw": np.ascontiguousarray(
                v_w[tp].reshape(16, 128, 256).transpose(1, 0, 2)).astype(NPB),
            "ow": np.ascontiguousarray(ow_np).astype(NPB),
            "cst": np.ascontiguousarray(cst_np),
            "masks": masks_np,
            "scs": scs_np,
        })

    res = run_bass_kernel_spmd(nc, in_maps, core_ids=list(range(8)))
    last_results = res

    out = np.zeros((B, T, D), np.float32)
    for core in range(8):
        out[core // 4] += np.asarray(res.results[core]["out"]).astype(np.float32).reshape(T, D)
    return out
